# revision 1
# baseline (speedup 1.0000x reference)
"""Trainium2 Bass kernel for DCNv2 modulated deformable conv + BN + ReLU.

Problem: x[4,128,128,128], 3x3 deformable conv (offsets/mask from a dense
3x3 conv), 1 deformable group, BN (inference) + ReLU.

Sharding: 8 cores = (batch b = core//2) x (row-half h = core%2).
Each core computes output rows [64h, 64h+64) of batch b.

Per-core pipeline:
  S1  transpose halo slab -> XT [x, y, c] bf16 in SBUF
  S2  write pair image XPD[y, x] = (pix(y,x), pix(y,x+1)) channels -> DRAM
  S3  offset conv (fp32r matmuls) -> om [27, pos]; PE-transpose -> OMT [pos, 27]
  S4  offset math on DVE -> slot-weight coefs (bf16) + gather indices (int16)
  S5  dma_gather pair rows from XPD (HBM, sample-major [pos, (2pix,c)])
  S6  bilinear combine: V = sum_q wq * plane_q   (DVE, bf16)
  S7  PE-transpose V -> [c, pos]; main matmul over (c,k) bf16 -> psum
  S8  ACT epilogue relu(psum*A + B) -> out rows -> DRAM
"""
import os
import numpy as np
import ml_dtypes
from contextlib import ExitStack

import concourse.bass as bass
import concourse.mybir as mybir
import concourse.tile as tile
from concourse import bacc
from concourse.masks import make_identity
from concourse import library_config

F32 = mybir.dt.float32
F32R = mybir.dt.float32r
BF16 = mybir.dt.bfloat16
I16 = mybir.dt.int16
I32 = mybir.dt.int32
AL = mybir.AluOpType
ACT = mybir.ActivationFunctionType

B, C, H, W = 4, 128, 128, 128
CO = 128
K2 = 9
HL = 88           # halo slab rows per core
RT = 64           # output rows per core
RB = 2            # rows per gather block
NBLK = RT // RB   # 32 blocks
GRP = RB * K2     # gather groups per block (18)
NK = RT * K2      # 576
EPS = 1e-5

_CACHE = {}


def _build_nc():
    nc = bacc.Bacc("TRN2", target_bir_lowering=False)

    # ---------------- I/O ----------------
    xp_d = nc.dram_tensor("xp", [C, 66 * 130], F32, kind="ExternalInput")
    xh_d = nc.dram_tensor("xh", [C, HL * W], F32, kind="ExternalInput")
    wom_d = nc.dram_tensor("wom", [C, K2 * 27], F32, kind="ExternalInput")
    bom_d = nc.dram_tensor("bom", [27, 1], F32, kind="ExternalInput")
    wl_d = nc.dram_tensor("wl", [C, K2 * CO], BF16, kind="ExternalInput")
    av_d = nc.dram_tensor("av", [CO, 1], F32, kind="ExternalInput")
    bv_d = nc.dram_tensor("bv", [CO, 1], F32, kind="ExternalInput")
    rk_d = nc.dram_tensor("rk", [128, NK], F32, kind="ExternalInput")   # 64h+r+ky-1
    kxx_d = nc.dram_tensor("kxx", [128, NK], F32, kind="ExternalInput")  # p+kx-1
    ybase_d = nc.dram_tensor("ybase", [128, 1], F32, kind="ExternalInput")

    yl_d = nc.dram_tensor("yl", [CO, RT * W], F32, kind="ExternalOutput")
    xpd_d = nc.dram_tensor("xpd", [HL * W, 256], BF16, kind="Internal")

    with ExitStack() as ctx:
        tc = ctx.enter_context(tile.TileContext(nc))
        cp = ctx.enter_context(tc.tile_pool(name="const", bufs=1))

        ep = ctx.enter_context(tc.tile_pool(name="early", bufs=1))
        # persistent tiles
        xt = ep.tile([128, HL * C], BF16)           # XT[x, y*128+c]
        omt = cp.tile([128, RT * 27], F32)          # OMT[p, r*27+ch]
        w00 = cp.tile([128, NK], BF16)
        w01 = cp.tile([128, NK], BF16)
        w10 = cp.tile([128, NK], BF16)
        w11 = cp.tile([128, NK], BF16)
        wr0 = cp.tile([128, NK * 8], I16)           # wrapped idx (y0 rows)
        wr1 = cp.tile([128, NK * 8], I16)           # wrapped idx (y1 rows)
        w_sb = cp.tile([128, K2 * CO], BF16)
        wom_sb = cp.tile([128, K2 * 27], F32)
        bom_sb = cp.tile([27, 1], F32)
        av_sb = cp.tile([CO, 1], F32)
        bv_sb = cp.tile([CO, 1], F32)
        rk_sb = cp.tile([128, NK], F32)
        kxx_sb = cp.tile([128, NK], F32)
        ybase_sb = cp.tile([128, 1], F32)
        idf = cp.tile([128, 128], F32)
        idb = cp.tile([128, 128], BF16)
        xp_sb = ep.tile([128, 66 * 130], F32)

        nc.gpsimd.load_library(library_config.mlp)
        nc.sync.dma_start(w_sb[:], wl_d[:])
        nc.sync.dma_start(wom_sb[:], wom_d[:])
        nc.sync.dma_start(bom_sb[:], bom_d[:])
        nc.sync.dma_start(av_sb[:], av_d[:])
        nc.sync.dma_start(bv_sb[:], bv_d[:])
        nc.sync.dma_start(rk_sb[:], rk_d[:])
        nc.sync.dma_start(kxx_sb[:], kxx_d[:])
        nc.sync.dma_start(ybase_sb[:], ybase_d[:])
        nc.sync.dma_start(xp_sb[:], xp_d[:])
        make_identity(nc, idf[:])
        make_identity(nc, idb[:])

        # ---------- S1: build XT (transpose halo slab, cast bf16) ----------
        with tc.tile_pool(name="s1", bufs=2) as s1, \
             tc.tile_pool(name="s1p", bufs=2, space="PSUM") as s1p:
            CH = 8
            for cidx in range(HL // CH):
                xh_sb = s1.tile([128, CH * W], F32, tag="xh")
                nc.sync.dma_start(xh_sb[:], xh_d[:, cidx * CH * W:(cidx + 1) * CH * W])
                for half in range(CH // 4):
                    pt = s1p.tile([128, 512], F32, tag="ptx")
                    for j in range(4):
                        row = half * 4 + j
                        nc.tensor.transpose(pt[:, j * 128:(j + 1) * 128],
                                            xh_sb[:, row * W:(row + 1) * W], idf[:])
                    y0r = cidx * CH + half * 4
                    nc.scalar.copy(xt[:, y0r * C:(y0r + 4) * C], pt[:])

        # ---------- S2: write XPD pair image to DRAM ----------
        xpd_4d = xpd_d.ap().rearrange("(y x) (s c) -> y x s c", x=128, s=2)
        xt_v = xt[:].rearrange("x (y c) -> x y c", y=HL)
        nc.sync.dma_start(xpd_4d[:, :, 0, :].rearrange("y x c -> x y c"), xt_v)
        nc.sync.dma_start(xpd_4d[:, 0:127, 1, :].rearrange("y x c -> x y c"),
                          xt_v[1:128])

        # ---------- S3: offset conv + OMT ----------
        xp_v = xp_sb[:].rearrange("c (r x) -> c r x", x=130)
        with tc.tile_pool(name="s3om", bufs=2) as s3om, \
             tc.tile_pool(name="s3po", bufs=2, space="PSUM") as s3po, \
             tc.tile_pool(name="s3pt", bufs=2, space="PSUM") as s3pt:
            for rb4 in range(RT // 4):
                pom = s3po.tile([27, 512], F32, tag="pom")
                for k in range(K2):
                    ky, kx = k // 3, k % 3
                    rhs = xp_v[:, rb4 * 4 + ky:rb4 * 4 + ky + 4, kx:kx + 128]
                    nc.tensor.matmul(pom[:].rearrange("o (r x) -> o r x", x=128),
                                     wom_sb[:, k * 27:(k + 1) * 27],
                                     rhs,
                                     start=(k == 0), stop=(k == K2 - 1))
                om_sb = s3om.tile([27, 512], F32, tag="om")
                nc.scalar.activation(om_sb[:], pom[:], ACT.Identity,
                                     bias=bom_sb[:], scale=1.0)
                pt = s3pt.tile([128, 108], F32, tag="pomt")
                for j in range(4):
                    nc.tensor.transpose(pt[:, j * 27:(j + 1) * 27],
                                        om_sb[:, j * 128:(j + 1) * 128],
                                        idf[0:27, 0:27])
                nc.scalar.copy(omt[:, rb4 * 108:(rb4 + 1) * 108], pt[:])

        # ---------- S4: offset math ----------
        with tc.tile_pool(name="s4", bufs=1) as s4:
            cnt = [0]

            def t():
                cnt[0] += 1
                return s4.tile([128, NK], F32, tag=f"s4_{cnt[0]}", name=f"s4_{cnt[0]}")

            omt_v = omt[:].rearrange("p (r ch) -> p r ch", ch=27)
            off18 = omt_v[:, :, 0:18].rearrange("p r (ch two) -> p r ch two", two=2)
            dy = off18[:, :, :, 0]
            dx = off18[:, :, :, 1]
            mm = omt_v[:, :, 18:27]

            def v3(ap):  # [128, NK] tile -> [128, RT, K2] view
                return ap[:].rearrange("p (r k) -> p r k", k=K2)

            py = t(); px = t()
            nc.vector.tensor_tensor(v3(py), dy, v3(rk_sb), AL.add)
            nc.vector.tensor_tensor(v3(px), dx, v3(kxx_sb), AL.add)

            def floor_(src):
                ti = s4.tile([128, NK], I32, tag=f"s4i_{cnt[0]}", name=f"s4i_{cnt[0]}")
                nc.vector.tensor_copy(ti[:], src[:])
                tr = t()
                nc.vector.tensor_copy(tr[:], ti[:])
                tcmp = t()
                nc.vector.tensor_tensor(tcmp[:], tr[:], src[:], AL.is_gt)
                out = t()
                nc.vector.tensor_tensor(out[:], tr[:], tcmp[:], AL.subtract)
                return out

            y0 = floor_(py)
            x0 = floor_(px)
            fy = t(); nc.vector.tensor_tensor(fy[:], py[:], y0[:], AL.subtract)
            fx = t(); nc.vector.tensor_tensor(fx[:], px[:], x0[:], AL.subtract)

            yb = t(); nc.vector.tensor_scalar(yb[:], y0[:], 126.0, 0.0, AL.min, AL.max)
            xb = t(); nc.vector.tensor_scalar(xb[:], x0[:], 126.0, 0.0, AL.min, AL.max)

            msk = t()
            nc.scalar.activation(v3(msk), mm, ACT.Sigmoid)

            def slots(v0, vb, f, mask):
                d = t(); nc.vector.tensor_tensor(d[:], v0[:], vb[:], AL.subtract)
                e0 = t(); nc.vector.tensor_scalar(e0[:], d[:], 0.0, None, AL.is_equal)
                em = t(); nc.vector.tensor_scalar(em[:], d[:], -1.0, None, AL.is_equal)
                ep = t(); nc.vector.tensor_scalar(ep[:], d[:], 1.0, None, AL.is_equal)
                cf = t(); nc.vector.tensor_scalar(cf[:], f[:], -1.0, 1.0, AL.mult, AL.add)
                w0 = t(); w1 = t()
                t1 = t(); nc.vector.tensor_tensor(t1[:], e0[:], cf[:], AL.mult)
                t2 = t(); nc.vector.tensor_tensor(t2[:], em[:], f[:], AL.mult)
                nc.vector.tensor_tensor(w0[:], t1[:], t2[:], AL.add)
                t3 = t(); nc.vector.tensor_tensor(t3[:], e0[:], f[:], AL.mult)
                t4 = t(); nc.vector.tensor_tensor(t4[:], ep[:], cf[:], AL.mult)
                nc.vector.tensor_tensor(w1[:], t3[:], t4[:], AL.add)
                if mask is not None:
                    nc.vector.tensor_tensor(w0[:], w0[:], mask[:], AL.mult)
                    nc.vector.tensor_tensor(w1[:], w1[:], mask[:], AL.mult)
                return w0, w1

            wy0, wy1 = slots(y0, yb, fy, msk)
            wx0, wx1 = slots(x0, xb, fx, None)

            nc.vector.tensor_tensor(w00[:], wy0[:], wx0[:], AL.mult)
            nc.vector.tensor_tensor(w01[:], wy0[:], wx1[:], AL.mult)
            nc.vector.tensor_tensor(w10[:], wy1[:], wx0[:], AL.mult)
            nc.vector.tensor_tensor(w11[:], wy1[:], wx1[:], AL.mult)

            # indices: idx0 = clamp(yb - ybase, 0, HL-2)*128 + xb
            ybl = t()
            nc.vector.tensor_scalar(ybl[:], yb[:], ybase_sb[:, 0:1], None, AL.subtract)
            nc.vector.tensor_scalar(ybl[:], ybl[:], float(HL - 2), 0.0, AL.min, AL.max)
            idxf = t()
            nc.vector.tensor_scalar(idxf[:], ybl[:], 128.0, None, AL.mult)
            nc.vector.tensor_tensor(idxf[:], idxf[:], xb[:], AL.add)
            idx0 = s4.tile([128, NK], I16, tag="idx0")
            idx1 = s4.tile([128, NK], I16, tag="idx1")
            nc.vector.tensor_copy(idx0[:], idxf[:])
            nc.vector.tensor_scalar(idxf[:], idxf[:], 128.0, None, AL.add)
            nc.vector.tensor_copy(idx1[:], idxf[:])

            # wrap-reorg: wr[16G+pp, g*8+a] = idx[16a+pp, g]  for all G
            for src, dst in ((idx0, wr0), (idx1, wr1)):
                dst_v = dst[:].rearrange("q (g a) -> q g a", a=8)
                for a in range(8):
                    nc.sync.dma_start(dst_v[0:16, :, a],
                                      src[16 * a:16 * (a + 1), :])
                for g in range(1, 8):
                    nc.sync.dma_start(dst[16 * g:16 * (g + 1), :], dst[0:16, :])

        # ---------- S5..S8: main loop ----------
        _stage = os.environ.get("DCN_STAGE", "full")
        if _stage != "front":
         with tc.tile_pool(name="mg", bufs=2) as mg, \
             tc.tile_pool(name="mv", bufs=2) as mv, \
             tc.tile_pool(name="mvt", bufs=2) as mvt, \
             tc.tile_pool(name="mo", bufs=2) as mo, \
             tc.tile_pool(name="mpv", bufs=3, space="PSUM") as mpv, \
             tc.tile_pool(name="mpo", bufs=2, space="PSUM") as mpo:
            OCH = 8  # output rows per store DMA
            out_sb = None
            for blk in range(NBLK):
                g0 = mg.tile([128, GRP, 256], BF16, tag="g0")
                g1 = mg.tile([128, GRP, 256], BF16, tag="g1")
                ni = GRP * 128
                s = blk * GRP * 8
                if _stage == "nogather":
                    nc.vector.memset(g0[:], 0.25)
                    nc.vector.memset(g1[:], 0.25)
                else:
                    nc.gpsimd.dma_gather(g0[:], xpd_d.ap(), wr0[:, s:s + GRP * 8],
                                         num_idxs=ni, num_idxs_reg=ni, elem_size=256,
                                         single_packet=False)
                    nc.gpsimd.dma_gather(g1[:], xpd_d.ap(), wr1[:, s:s + GRP * 8],
                                         num_idxs=ni, num_idxs_reg=ni, elem_size=256,
                                         single_packet=False)

                # combine: V = w00*g0A + w01*g0B + w10*g1A + w11*g1B
                # coefs pre-expanded 8-wide so every operand's innermost AP dim
                # is step-1 (unlocks DVE 2x bf16 mode; stride-0 goes to a mid dim)
                V = mv.tile([128, GRP, 128], BF16, tag="V")
                tmp = mv.tile([128, GRP, 128], BF16, tag="Vtmp")
                ce = [mv.tile([128, GRP, 8], BF16, tag=f"ce{i}", name=f"ce{i}")
                      for i in range(4)]
                for i, wt in enumerate((w00, w01, w10, w11)):
                    nc.vector.tensor_copy(
                        ce[i][:], wt[:, blk * GRP:(blk + 1) * GRP].unsqueeze(-1)
                        .broadcast_to((128, GRP, 8)))

                def coefx(i):
                    return ce[i][:].unsqueeze(2).broadcast_to((128, GRP, 16, 8))

                def plane(g, sl):
                    v = g[:].rearrange("p g (s ch cl) -> p g s ch cl", s=2, cl=8)
                    return v[:, :, sl, :, :]

                def v4(ap):
                    return ap.rearrange("p g (ch cl) -> p g ch cl", cl=8)

                nc.vector.tensor_tensor(v4(V[:]), plane(g0, 0), coefx(0), AL.mult)
                nc.vector.tensor_tensor(v4(tmp[:]), plane(g0, 1), coefx(1), AL.mult)
                nc.vector.tensor_tensor(V[:], V[:], tmp[:], AL.add)
                nc.vector.tensor_tensor(v4(tmp[:]), plane(g1, 0), coefx(2), AL.mult)
                nc.vector.tensor_tensor(V[:], V[:], tmp[:], AL.add)
                nc.vector.tensor_tensor(v4(tmp[:]), plane(g1, 1), coefx(3), AL.mult)
                nc.vector.tensor_tensor(V[:], V[:], tmp[:], AL.add)

                # transpose V -> VT [c, (rr,k)*128]
                vt = mvt.tile([128, GRP * 128], BF16, tag="VT")
                for h4 in range((GRP + 3) // 4):
                    pvt = mpv.tile([128, 512], BF16, tag="pvt")
                    n4 = min(4, GRP - h4 * 4)
                    for j in range(n4):
                        g = h4 * 4 + j
                        nc.tensor.transpose(pvt[:, j * 128:(j + 1) * 128],
                                            V[:, g, :], idb[:])
                    nc.scalar.copy(vt[:, h4 * 512:h4 * 512 + n4 * 128],
                                   pvt[:, 0:n4 * 128])

                # main matmul + epilogue
                if blk % (OCH // RB) == 0:
                    out_sb = mo.tile([128, OCH * W], F32, tag="osb")
                for rr in range(RB):
                    po = mpo.tile([128, 128], F32, tag="po")
                    for k in range(K2):
                        g = rr * K2 + k
                        nc.tensor.matmul(po[:], w_sb[:, k * CO:(k + 1) * CO],
                                         vt[:, g * 128:(g + 1) * 128],
                                         start=(k == 0), stop=(k == K2 - 1))
                    ro = (blk * RB + rr) % OCH
                    nc.scalar.activation(out_sb[:, ro * W:(ro + 1) * W], po[:],
                                         ACT.Relu, bias=bv_sb[:], scale=av_sb[:])
                if (blk * RB + RB) % OCH == 0:
                    r0 = (blk * RB + RB) - OCH
                    nc.sync.dma_start(yl_d[:, r0 * W:(r0 + OCH) * W], out_sb[:])

    nc.compile()
    return nc


def _prep_inputs(x, w_om, b_om, w, b, gamma, beta, bn_mean, bn_var):
    """Build the 8 per-core input maps."""
    x = np.ascontiguousarray(x, dtype=np.float32)
    A = (gamma / np.sqrt(bn_var + EPS)).astype(np.float32)
    Bv = ((b - bn_mean) * A + beta).astype(np.float32)
    wom_l = np.ascontiguousarray(
        w_om.reshape(27, C, K2).transpose(1, 2, 0)).astype(np.float32).reshape(C, K2 * 27)
    wl = np.ascontiguousarray(
        w.reshape(CO, C, K2).transpose(1, 2, 0)).astype(ml_dtypes.bfloat16).reshape(C, K2 * CO)
    r = np.arange(RT, dtype=np.float32)[:, None]
    kyv = (np.arange(K2, dtype=np.float32) // 3)[None, :]
    kxv = (np.arange(K2, dtype=np.float32) % 3)[None, :]
    p = np.arange(128, dtype=np.float32)[:, None, None]
    kxx = (np.broadcast_to((kxv - 1)[None], (128, RT, K2))
           + np.broadcast_to(p, (128, RT, K2))).reshape(128, NK).astype(np.float32)
    in_maps = []
    for core in range(8):
        bidx, h = core // 2, core % 2
        ylo = 0 if h == 0 else H - HL
        xp = np.zeros((C, 66, 130), np.float32)
        r0 = 64 * h - 1
        rlo, rhi = max(r0, 0), min(r0 + 66, H)
        xp[:, rlo - r0:rhi - r0, 1:129] = x[bidx, :, rlo:rhi, :]
        xh = np.ascontiguousarray(x[bidx, :, ylo:ylo + HL, :])
        rk = np.broadcast_to((64 * h + r + kyv - 1)[None],
                             (128, RT, K2)).reshape(128, NK)
        in_maps.append(dict(
            xp=np.ascontiguousarray(xp.reshape(C, 66 * 130)),
            xh=xh.reshape(C, HL * W),
            wom=wom_l, bom=b_om.reshape(27, 1).astype(np.float32),
            wl=wl, av=A.reshape(CO, 1), bv=Bv.reshape(CO, 1),
            rk=np.ascontiguousarray(rk, dtype=np.float32),
            kxx=kxx,
            ybase=np.full((128, 1), ylo, np.float32),
        ))
    return in_maps


def kernel(x, w_om, b_om, w, b, gamma, beta, bn_mean, bn_var):
    from concourse.bass_utils import run_bass_kernel_spmd
    if "nc" not in _CACHE:
        _CACHE["nc"] = _build_nc()
    nc = _CACHE["nc"]
    in_maps = _prep_inputs(x, w_om, b_om, w, b, gamma, beta, bn_mean, bn_var)
    res = run_bass_kernel_spmd(nc, in_maps, core_ids=list(range(8)),
                               trace=bool(int(os.environ.get("DCN_TRACE", "0"))))
    out = np.zeros((B, CO, H, W), np.float32)
    for core in range(8):
        bidx, h = core // 2, core % 2
        out[bidx, :, 64 * h:64 * h + 64, :] = \
            res.results[core]["yl"].reshape(CO, RT, W)
    _CACHE["last_result"] = res
    return out



# revision 26
# speedup vs baseline: 1.9279x; 1.9279x over previous
"""Trainium2 Bass kernel for DCNv2 modulated deformable conv + BN + ReLU.

Problem: x[4,128,128,128], 3x3 deformable conv (offsets/mask from a dense
3x3 conv), 1 deformable group, BN (inference) + ReLU.

Sharding: 8 cores = (batch b = core//2) x (row-half h = core%2).
Each core computes output rows [64h, 64h+64) of batch b.

v2 pipeline (per core), software-pipelined over superblocks of rows
(8,8,16,16,16):
  host      quad image XQ[(y,x), 4px*c] bf16 in DRAM, xp padded f32
  front(sb) offset conv (f32r matmuls) -> om; PE-transpose -> OMT;
            offset math on DVE (f32) -> 4 corner-weight tiles (bf16,
            8-wide expanded on ACT) + quad gather indices; index
            wrap-reorg built with f32 selection matmuls on PE
  main(sb)  per 2-row block:
            dma_gather quad elems (1KB each) from XQ
            4 DVE mults apply corner weights in place
            per row: 36 PSUM-accumulating PE transposes -> VT row tile
            ACT copies VT psum->sbuf (k-major)
            9 accumulating matmuls (256-col) over k -> psum
            ACT epilogue relu(psum*A + B) -> out rows -> DRAM
"""
import os
import numpy as np
import ml_dtypes
from contextlib import ExitStack

import concourse.bass as bass
import concourse.mybir as mybir
import concourse.tile as tile
from concourse import bacc
from concourse.masks import make_identity
from concourse import library_config

F32 = mybir.dt.float32
F32R = mybir.dt.float32r
F16 = mybir.dt.float16
BF16 = mybir.dt.bfloat16
I16 = mybir.dt.int16
I32 = mybir.dt.int32
AL = mybir.AluOpType
ACT = mybir.ActivationFunctionType

B, C, H, W = 4, 128, 128, 128
CO = 128
K2 = 9
HL = 88           # halo slab rows per core
RT = 64           # output rows per core
RB = 2            # rows per gather block
GRP = RB * K2     # sample groups per block (18)
NK = RT * K2      # 576
SBS = [8, 8, 16, 16, 16]          # superblock row counts
SB0 = [sum(SBS[:i]) for i in range(len(SBS))]  # row starts
SBKM = max(SBS) * K2              # max offset-math cols (144)
EPS = 1e-5

_CACHE = {}


def _build_nc(mode="T4"):
    nc = bacc.Bacc("TRN2", target_bir_lowering=False)

    # ---------------- I/O ----------------
    xq_d = nc.dram_tensor("xq", [HL * W, 512], BF16, kind="ExternalInput")
    xp_d = nc.dram_tensor("xp", [C, 66 * 130], F16, kind="ExternalInput")
    wom_d = nc.dram_tensor("wom", [C, K2 * 27], F16, kind="ExternalInput")
    bom_d = nc.dram_tensor("bom", [27, 1], F32, kind="ExternalInput")
    wl_d = nc.dram_tensor("wl", [C, K2 * CO], BF16, kind="ExternalInput")
    av_d = nc.dram_tensor("av", [CO, 1], F32, kind="ExternalInput")
    bv_d = nc.dram_tensor("bv", [CO, 1], F32, kind="ExternalInput")
    rk_d = nc.dram_tensor("rk", [128, NK], F32, kind="ExternalInput")   # 64h+r+ky-1
    kxx_d = nc.dram_tensor("kxx", [128, NK], F32, kind="ExternalInput")  # p+kx-1
    ybase_d = nc.dram_tensor("ybase", [128, 1], F32, kind="ExternalInput")
    sel_d = nc.dram_tensor("sel", [128, 8 * 128], F32, kind="ExternalInput")

    yl_d = nc.dram_tensor("yl", [CO, RT * W], F32, kind="ExternalOutput")

    with ExitStack() as ctx:
        tc = ctx.enter_context(tile.TileContext(nc))
        cp = ctx.enter_context(tc.tile_pool(name="const", bufs=1))

        w_sb = cp.tile([128, K2 * CO], BF16)
        wom_sb = cp.tile([128, K2 * 27], F16)
        bom_sb = cp.tile([27, 1], F32)
        av_sb = cp.tile([CO, 1], F32)
        bv_sb = cp.tile([CO, 1], F32)
        rk_sb = cp.tile([128, NK], F32)
        kxx_sb = cp.tile([128, NK], F32)
        ybase_sb = cp.tile([128, 1], F32)
        sel_sb = cp.tile([128, 8 * 128], F32)
        idf = cp.tile([128, 128], F32)
        idb = cp.tile([128, 128], BF16)
        # xp loaded in per-superblock overlapping row chunks so front(0)
        # is not gated by one monolithic load
        xp4 = [cp.tile([128, (nr + 2) * 130], F16, tag=f"xp{sb}",
                       name=f"xp{sb}")
               for sb, nr in enumerate(SBS)]

        nc.gpsimd.load_library(library_config.mlp)
        xp_dv = xp_d.ap().rearrange("c (r x) -> c r x", x=130)
        for sb, nr in enumerate(SBS):
            nc.sync.dma_start(xp4[sb][:],
                              xp_dv[:, SB0[sb]:SB0[sb] + nr + 2, :]
                              .rearrange("c r x -> c (r x)"))
        nc.sync.dma_start(wom_sb[:], wom_d[:])
        nc.sync.dma_start(w_sb[:], wl_d[:])
        nc.sync.dma_start(bom_sb[:], bom_d[:])
        nc.sync.dma_start(av_sb[:], av_d[:])
        nc.sync.dma_start(bv_sb[:], bv_d[:])
        nc.sync.dma_start(rk_sb[:], rk_d[:])
        nc.sync.dma_start(kxx_sb[:], kxx_d[:])
        nc.sync.dma_start(ybase_sb[:], ybase_d[:])
        nc.sync.dma_start(sel_sb[:], sel_d[:])
        make_identity(nc, idf[:])
        make_identity(nc, idb[:])

        xp4_v = [t[:].rearrange("c (r x) -> c r x", x=130) for t in xp4]

        fp = ctx.enter_context(tc.tile_pool(name="fomt", bufs=2))
        fom = ctx.enter_context(tc.tile_pool(name="fom", bufs=2))
        fce = ctx.enter_context(tc.tile_pool(name="fce", bufs=2))
        fwr = ctx.enter_context(tc.tile_pool(name="fwr", bufs=2))
        fs4 = ctx.enter_context(tc.tile_pool(name="fs4", bufs=2))
        fps = ctx.enter_context(tc.tile_pool(name="fps", bufs=1, space="PSUM"))
        mps = ctx.enter_context(tc.tile_pool(name="mps", bufs=2, space="PSUM"))

        def front(sb):
            """Offset conv + offset math for rows [SB0[sb], +SBS[sb])."""
            nr = SBS[sb]
            w = nr * K2
            # ---- offset conv -> omt [p, r*27+ch] ----
            omt = fp.tile([128, SBKM * 3], F32, tag="omt", name=f"omt{sb}")
            for i2 in range(nr // 2):
                pomt = fps.tile([128, 512], F32, tag="fscr", bufs=1,
                                name="pomt")
                pom = pomt[0:27, 0:256]
                for k in range(K2):
                    ky, kx = k // 3, k % 3
                    rhs = xp4_v[sb][:, i2 * 2 + ky:i2 * 2 + ky + 2, kx:kx + 128]
                    nc.tensor.matmul(pom.rearrange("o (r x) -> o r x", x=128),
                                     wom_sb[:, k * 27:(k + 1) * 27],
                                     rhs,
                                     start=(k == 0), stop=(k == K2 - 1))
                om_sb = fom.tile([27, 256], F32, tag="om", name="om_sb")
                nc.scalar.activation(om_sb[:], pom, ACT.Identity,
                                     bias=bom_sb[:], scale=1.0)
                pt = fps.tile([128, 512], F32, tag="fscr", bufs=1, name="pt")
                for j in range(2):
                    nc.tensor.transpose(pt[:, j * 27:(j + 1) * 27],
                                        om_sb[:, j * 128:(j + 1) * 128],
                                        idf[0:27, 0:27])
                nc.scalar.copy(omt[:, i2 * 54:(i2 + 1) * 54], pt[:, 0:54])

            # ---- offset math (on [128, w] slices) ----
            cnt = [0]

            def t():
                cnt[0] += 1
                full = fs4.tile([128, SBKM], F32, tag=f"s4_{cnt[0]}",
                                name=f"s4_{cnt[0]}_{sb}")
                return full

            def s(ap):  # active slice
                return ap[:, 0:w]

            omt_v = omt[:, 0:nr * 27].rearrange("p (r ch) -> p r ch", ch=27)
            off18 = omt_v[:, :, 0:18].rearrange("p r (ch two) -> p r ch two",
                                                two=2)
            dy = off18[:, :, :, 0]
            dx = off18[:, :, :, 1]
            mm = omt_v[:, :, 18:27]

            def v3(ap):  # [128, w] slice -> [128, nr, K2] view
                return ap[:, 0:w].rearrange("p (r k) -> p r k", k=K2)

            py = t(); px = t()
            nc.vector.tensor_tensor(v3(py), dy,
                                    v3(rk_sb[:, SB0[sb] * K2:]), AL.add)
            nc.vector.tensor_tensor(v3(px), dx,
                                    v3(kxx_sb[:, SB0[sb] * K2:]), AL.add)

            def floor_(src):
                ti = fs4.tile([128, SBKM], I32, tag=f"s4i_{cnt[0]}",
                              name=f"s4i_{cnt[0]}_{sb}")
                nc.vector.tensor_copy(ti[:, 0:w], s(src))
                tr = t()
                nc.vector.tensor_copy(s(tr), ti[:, 0:w])
                tcmp = t()
                nc.vector.tensor_tensor(s(tcmp), s(tr), s(src), AL.is_gt)
                out = t()
                nc.vector.tensor_tensor(s(out), s(tr), s(tcmp), AL.subtract)
                return out

            y0 = floor_(py)
            x0 = floor_(px)
            fy = t(); nc.vector.tensor_tensor(s(fy), s(py), s(y0), AL.subtract)
            fx = t(); nc.vector.tensor_tensor(s(fx), s(px), s(x0), AL.subtract)

            yb = t(); nc.vector.tensor_scalar(s(yb), s(y0), 126.0, 0.0,
                                              AL.min, AL.max)
            xb = t(); nc.vector.tensor_scalar(s(xb), s(x0), 126.0, 0.0,
                                              AL.min, AL.max)

            msk = t()
            nc.scalar.activation(v3(msk), mm, ACT.Sigmoid)

            def slots(v0, vb, f, mask):
                d = t(); nc.vector.tensor_tensor(s(d), s(v0), s(vb), AL.subtract)
                e0 = t(); nc.vector.tensor_scalar(s(e0), s(d), 0.0, None,
                                                  AL.is_equal)
                em = t(); nc.vector.tensor_scalar(s(em), s(d), -1.0, None,
                                                  AL.is_equal)
                ep = t(); nc.vector.tensor_scalar(s(ep), s(d), 1.0, None,
                                                  AL.is_equal)
                cf = t(); nc.vector.tensor_scalar(s(cf), s(f), -1.0, 1.0,
                                                  AL.mult, AL.add)
                w0 = t(); w1 = t()
                t1 = t(); nc.vector.tensor_tensor(s(t1), s(e0), s(cf), AL.mult)
                t2 = t(); nc.vector.tensor_tensor(s(t2), s(em), s(f), AL.mult)
                nc.vector.tensor_tensor(s(w0), s(t1), s(t2), AL.add)
                t3 = t(); nc.vector.tensor_tensor(s(t3), s(e0), s(f), AL.mult)
                t4 = t(); nc.vector.tensor_tensor(s(t4), s(ep), s(cf), AL.mult)
                nc.vector.tensor_tensor(s(w1), s(t3), s(t4), AL.add)
                if mask is not None:
                    nc.vector.tensor_tensor(s(w0), s(w0), s(mask), AL.mult)
                    nc.vector.tensor_tensor(s(w1), s(w1), s(mask), AL.mult)
                return w0, w1

            wy0, wy1 = slots(y0, yb, fy, msk)
            wx0, wx1 = slots(x0, xb, fx, None)

            wq = [fs4.tile([128, SBKM], BF16, tag=f"wq{i}", name=f"wq{i}_{sb}")
                  for i in range(4)]
            nc.vector.tensor_tensor(wq[0][:, 0:w], s(wy0), s(wx0), AL.mult)
            nc.vector.tensor_tensor(wq[1][:, 0:w], s(wy0), s(wx1), AL.mult)
            nc.vector.tensor_tensor(wq[2][:, 0:w], s(wy1), s(wx0), AL.mult)
            nc.vector.tensor_tensor(wq[3][:, 0:w], s(wy1), s(wx1), AL.mult)
            # 8-wide expansion for the combine's stride-1 bf16 AP (on ACT)
            ce = [fce.tile([128, SBKM, 8], BF16, tag=f"cq{i}",
                           name=f"cq{i}_{sb}")
                  for i in range(4)]
            for i in range(4):
                nc.scalar.copy(ce[i][:, 0:w, :], wq[i][:, 0:w].unsqueeze(-1)
                               .broadcast_to((128, w, 8)))

            # quad gather index: idx = clamp(yb - ybase, 0, HL-2)*128 + xb
            ybl = t()
            nc.vector.tensor_scalar(s(ybl), s(yb), ybase_sb[:, 0:1], None,
                                    AL.subtract)
            nc.vector.tensor_scalar(s(ybl), s(ybl), float(HL - 2), 0.0,
                                    AL.min, AL.max)
            idxf = t()
            nc.vector.tensor_scalar(s(idxf), s(ybl), 128.0, None, AL.mult)
            nc.vector.tensor_tensor(s(idxf), s(idxf), s(xb), AL.add)

            # wrap-reorg via f32 selection matmuls (index values are exact):
            # wr[s, g*8+a] = sum_p sel_a[p, s]*idxf[p, g] = idxf[16a+s%16, g]
            wrq = fwr.tile([128, SBKM * 8], I16, tag="wrq", name=f"wrq{sb}")
            wrq_v = wrq[:, 0:w * 8].rearrange("p (g a) -> p g a", a=8)
            for ar in range(4):
                wr_ps = fps.tile([128, 512], F32, tag="fscr", bufs=1,
                                 name="wr_ps")
                wr_v = wr_ps[:].rearrange("p (a g) -> p a g", a=2)
                for ai in range(2):
                    a = ar * 2 + ai
                    nc.tensor.matmul(wr_v[:, ai, 0:w],
                                     sel_sb[:, a * 128:(a + 1) * 128],
                                     s(idxf), start=True, stop=True)
                nc.vector.tensor_copy(
                    wrq_v[:, :, ar * 2:ar * 2 + 2],
                    wr_v[:, :, 0:w].rearrange("p a g -> p g a"))
            return ce, wrq

        # ---------- main loop over a superblock ----------
        _stage = os.environ.get("DCN_STAGE", "full")
        mg = ctx.enter_context(tc.tile_pool(name="mg", bufs=3))
        mvt = ctx.enter_context(tc.tile_pool(name="mvt", bufs=2))
        mo = ctx.enter_context(tc.tile_pool(name="mo", bufs=2))
        OCH = 8  # output rows per store DMA
        ni = GRP * 128
        state = {"out_sb": None}

        def main(sb, ce, wrq):
            for lb in range(SBS[sb] // RB):
                blk = SB0[sb] // RB + lb
                g = mg.tile([128, GRP, 512], BF16, tag="g", name="g")
                if _stage == "nogather":
                    nc.vector.memset(g[:], 0.25)
                else:
                    nc.gpsimd.dma_gather(g[:], xq_d.ap(),
                                         wrq[:, lb * GRP * 8:(lb + 1) * GRP * 8],
                                         num_idxs=ni, num_idxs_reg=ni,
                                         elem_size=512, single_packet=False)

                gv = g[:].rearrange("p g (q ch cl) -> p g q ch cl", q=4, cl=8)
                for q in range(4):
                    cx = ce[q][:, lb * GRP:(lb + 1) * GRP, :] \
                        .unsqueeze(2).broadcast_to((128, GRP, 16, 8))
                    nc.vector.tensor_tensor(gv[:, :, q], gv[:, :, q], cx,
                                            AL.mult)
                if mode != "T4":  # T1: sum planes on DVE
                    gq = g[:].rearrange("p g (q c) -> p g q c", q=4)
                    nc.vector.tensor_tensor(gq[:, :, 0], gq[:, :, 0],
                                            gq[:, :, 1], AL.add)
                    nc.vector.tensor_tensor(gq[:, :, 2], gq[:, :, 2],
                                            gq[:, :, 3], AL.add)
                    nc.vector.tensor_tensor(gq[:, :, 0], gq[:, :, 0],
                                            gq[:, :, 2], AL.add)

                # per-row "transposes" via regular accumulating matmuls
                # (data stationary x identity moving; transpose-mode PSUM
                # accumulation is broken on real HW), then one ACT copy
                vt = mvt.tile([128, GRP * 128], BF16, tag="vt", name="vt")
                vt_v = vt[:].rearrange("p (k r c) -> p k r c", k=K2, r=RB)
                qr = range(4) if mode == "T4" else (0,)
                for rr in range(RB):
                    vt_ps = mps.tile([128, K2 * 128], F32, tag="vtps",
                                     name="vt_ps")
                    for k in range(K2):
                        gi = rr * K2 + k
                        for q in qr:
                            nc.tensor.matmul(
                                vt_ps[:, k * 128:(k + 1) * 128],
                                g[:, gi, q * 128:(q + 1) * 128],
                                idb[:],
                                start=(q == qr[0]), stop=(q == qr[-1]))
                    nc.scalar.copy(vt_v[:, :, rr, :],
                                   vt_ps[:].rearrange("p (k c) -> p k c",
                                                      k=K2))

                # main matmul over k, 256-col rhs; po bufs=1 is free:
                # 72 vt-matmuls separate consecutive po chains on PE
                po = mps.tile([128, RB * 128], F32, tag="po", bufs=1,
                              name="po")
                for k in range(K2):
                    nc.tensor.matmul(po[:], w_sb[:, k * CO:(k + 1) * CO],
                                     vt[:, k * RB * 128:(k + 1) * RB * 128],
                                     start=(k == 0), stop=(k == K2 - 1))

                if blk % (OCH // RB) == 0:
                    state["out_sb"] = mo.tile([128, OCH * W], F32, tag="osb",
                                              name="osb")
                out_sb = state["out_sb"]
                ro = (blk * RB) % OCH
                nc.scalar.activation(out_sb[:, ro * W:(ro + RB) * W], po[:],
                                     ACT.Relu, bias=bv_sb[:], scale=av_sb[:])
                if (blk * RB + RB) % OCH == 0:
                    r0 = (blk * RB + RB) - OCH
                    nc.sync.dma_start(yl_d[:, r0 * W:(r0 + OCH) * W], out_sb[:])

        # software pipeline: front(0), front(1), main(0), front(2), ...
        fr = [front(0), front(1)]
        for sb in range(len(SBS)):
            if sb + 2 < len(SBS):
                fr.append(front(sb + 2))
            if _stage != "front":
                main(sb, *fr[sb])

    nc.compile()
    return nc


def _prep_inputs(x, w_om, b_om, w, b, gamma, beta, bn_mean, bn_var):
    """Build the 8 per-core input maps (all heavy layout work on host)."""
    x = np.ascontiguousarray(x, dtype=np.float32)
    A = (gamma / np.sqrt(bn_var + EPS)).astype(np.float32)
    Bv = ((b - bn_mean) * A + beta).astype(np.float32)
    wom_l = np.ascontiguousarray(
        w_om.reshape(27, C, K2).transpose(1, 2, 0)).astype(
            np.float16).reshape(C, K2 * 27)
    wl = np.ascontiguousarray(
        w.reshape(CO, C, K2).transpose(1, 2, 0)).astype(
            ml_dtypes.bfloat16).reshape(C, K2 * CO)
    r = np.arange(RT, dtype=np.float32)[:, None]
    kyv = (np.arange(K2, dtype=np.float32) // 3)[None, :]
    kxv = (np.arange(K2, dtype=np.float32) % 3)[None, :]
    p = np.arange(128, dtype=np.float32)[:, None, None]
    kxx = (np.broadcast_to((kxv - 1)[None], (128, RT, K2))
           + np.broadcast_to(p, (128, RT, K2))).reshape(128, NK).astype(np.float32)
    # selection matrices for the index wrap: sel_a[p, s] = (p == 16a + s%16)
    sel = np.zeros((128, 8, 128), np.float32)
    for a in range(8):
        for si in range(128):
            sel[16 * a + (si % 16), a, si] = 1.0
    sel = np.ascontiguousarray(sel.reshape(128, 8 * 128))

    in_maps = []
    for core in range(8):
        bidx, h = core // 2, core % 2
        ylo = 0 if h == 0 else H - HL
        # offset-conv input: rows r0..r0+65 padded 1 col each side
        xp = np.zeros((C, 66, 130), np.float32)
        r0 = 64 * h - 1
        rlo, rhi = max(r0, 0), min(r0 + 66, H)
        xp[:, rlo - r0:rhi - r0, 1:129] = x[bidx, :, rlo:rhi, :]
        # quad image: XQ[(yl,xc), qpix*128 + c]
        nrow = min(HL + 1, H - ylo)
        base = np.zeros((HL + 1, W + 1, C), np.float32)
        base[:nrow, :W, :] = x[bidx, :, ylo:ylo + nrow, :].transpose(1, 2, 0)
        xq = np.empty((HL, W, 4, C), np.float32)
        xq[:, :, 0] = base[:HL, :W]
        xq[:, :, 1] = base[:HL, 1:]
        xq[:, :, 2] = base[1:, :W]
        xq[:, :, 3] = base[1:, 1:]
        rk = np.broadcast_to((64 * h + r + kyv - 1)[None],
                             (128, RT, K2)).reshape(128, NK)
        in_maps.append(dict(
            xq=np.ascontiguousarray(
                xq.reshape(HL * W, 512).astype(ml_dtypes.bfloat16)),
            xp=np.ascontiguousarray(
                xp.reshape(C, 66 * 130).astype(np.float16)),
            wom=wom_l, bom=b_om.reshape(27, 1).astype(np.float32),
            wl=wl, av=A.reshape(CO, 1), bv=Bv.reshape(CO, 1),
            rk=np.ascontiguousarray(rk, dtype=np.float32),
            kxx=kxx,
            ybase=np.full((128, 1), ylo, np.float32),
            sel=sel,
        ))
    return in_maps


def kernel(x, w_om, b_om, w, b, gamma, beta, bn_mean, bn_var):
    from concourse.bass_utils import run_bass_kernel_spmd
    if "nc" not in _CACHE:
        _CACHE["nc"] = _build_nc(os.environ.get("DCN_MODE", "T4"))
    nc = _CACHE["nc"]
    in_maps = _prep_inputs(x, w_om, b_om, w, b, gamma, beta, bn_mean, bn_var)
    res = run_bass_kernel_spmd(nc, in_maps, core_ids=list(range(8)),
                               trace=bool(int(os.environ.get("DCN_TRACE", "0"))))
    out = np.zeros((B, CO, H, W), np.float32)
    for core in range(8):
        bidx, h = core // 2, core % 2
        out[bidx, :, 64 * h:64 * h + 64, :] = \
            res.results[core]["yl"].reshape(CO, RT, W)
    _CACHE["last_result"] = res
    return out
